# revision 5
# baseline (speedup 1.0000x reference)
"""Trainium2 Bass kernel for nn_ConvolutionEncoder_PWS (retrieval_knn).

kernel(**inputs) takes the FULL unsharded inputs (as produced by
reference.setup_inputs()) and returns the full (h, input_pws, hidden_pws)
tuple, distributing work across 8 NeuronCores internally.

Per-core plan (batch sharded 512 rows/core):
  1. indirect-DMA gather of embedding rows (tokens in l-major order),
     max_norm=1 renorm, PE-transpose into resident SBUF ET[384, 28*512]
     (channel-major, f32r) — feeds both the conv backbone and the gram.
  2. ET dumped to DRAM as [28, 384, 512] and AllGather'd across 8 cores.
  3. conv1/conv2/conv3 as PSUM-accumulated matmuls over (tap, cin-tile);
     training-mode BN via per-channel sums + AllReduce; BN+ReLU fused
     into a single ACT op on activation reload; tanh on ACT.
  4. hidden: row-normalize via PE-transpose round trip, AllGather [100,512],
     gram_h row-block [512, 4096].
  5. gram_in row-block [512, 4096]: 84 K-tiles x 8 column blocks, rhs
     streamed from the gathered ET, f32r matmuls at full PE rate.
Host side: input layout prep (weight transposes, token reordering) and
final assembly (vstack of row blocks, triu extraction).
"""

import sys

sys.path.insert(0, "/opt/trn_rl_repo")

import numpy as np

import concourse.bass as bass
import concourse.mybir as mybir
import concourse.tile as tile
from concourse import bacc
from concourse import bass_utils
from concourse.masks import make_identity

F32 = mybir.dt.float32
F32R = mybir.dt.float32r
I32 = mybir.dt.int32
AF = mybir.ActivationFunctionType
ALU = mybir.AluOpType

EPS_BN = 1e-5
EPS_RN = 1e-7


class Cfg:
    def __init__(self, B=4096, V=32000, ncores=8):
        self.B = B
        self.V = V
        self.ncores = ncores
        self.L = 28
        self.D = 300
        self.DP = 384          # padded D (3 x 128)
        self.NDT = 3           # d tiles
        self.BL = B // ncores  # batch rows per core
        assert self.BL % 128 == 0
        self.MB = self.BL // 128          # output row tiles per core
        self.NTOK = self.BL * self.L      # tokens per core
        self.NT = self.NTOK // 128        # gather tiles per core
        self.C1, self.C1P, self.NC1 = 300, 384, 3
        self.C2, self.C2P, self.NC2 = 600, 640, 5
        self.C3 = 100
        self.T1, self.T2 = 13, 5
        self.N1 = B * self.T1             # BN1 count
        self.N2 = B * self.T2             # BN2 count


def build(cfg: Cfg):
    nc = bacc.Bacc("TRN2", target_bir_lowering=False, debug=False,
                   num_devices=cfg.ncores)
    L, BL, DP, NDT = cfg.L, cfg.BL, cfg.DP, cfg.NDT
    NT, MB = cfg.NT, cfg.MB
    C1P, NC1, C2P, NC2, C3 = cfg.C1P, cfg.NC1, cfg.C2P, cfg.NC2, cfg.C3
    T1, T2 = cfg.T1, cfg.T2
    NCR = cfg.ncores
    RG = [list(range(NCR))]

    # ---- kernel I/O ----
    x_til = nc.dram_tensor("x_til", [128, NT], I32, kind="ExternalInput")
    emb = nc.dram_tensor("emb", [cfg.V, cfg.D], F32, kind="ExternalInput")
    w1t = nc.dram_tensor("w1t", [128, 5, NC1, C1P], F32R, kind="ExternalInput")
    w2t = nc.dram_tensor("w2t", [128, 5, NC1, C2P], F32R, kind="ExternalInput")
    w3t = nc.dram_tensor("w3t", [128, 5, NC2, C3], F32R, kind="ExternalInput")
    g1b1 = nc.dram_tensor("g1b1", [NC1, 128, 2], F32, kind="ExternalInput")
    g2b2 = nc.dram_tensor("g2b2", [NC2, 128, 2], F32, kind="ExternalInput")
    b3 = nc.dram_tensor("b3", [C3, 1], F32, kind="ExternalInput")

    g_in_blk = nc.dram_tensor("g_in_blk", [BL, cfg.B], F32, kind="ExternalOutput")
    g_h_blk = nc.dram_tensor("g_h_blk", [BL, cfg.B], F32, kind="ExternalOutput")
    h_out = nc.dram_tensor("h_out", [BL, C3], F32, kind="ExternalOutput")

    with tile.TileContext(nc) as tc:
        with (
            tc.tile_pool(name="persist", bufs=1) as pp,
            tc.tile_pool(name="dram", bufs=1, space="DRAM") as dr,
        ):
            # ---- persistent tiles ----
            et = [pp.tile([128, cfg.NTOK], F32R, tag=f"et{i}", name=f"et{i}")
                  for i in range(NDT)]
            ident = pp.tile([128, 128], F32, tag="ident", name="ident")
            make_identity(nc, ident[:])
            idx_sb = pp.tile([128, NT], I32, tag="idx", name="idx_sb")
            nc.sync.dma_start(idx_sb[:], x_til[:])
            fnT = pp.tile([128, BL], F32R, tag="fnT", name="fnT")
            # BN scale/shift tiles
            a1t = [pp.tile([128, 1], F32, tag=f"a1{i}", name=f"a1{i}") for i in range(NC1)]
            b1t = [pp.tile([128, 1], F32, tag=f"b1{i}", name=f"b1{i}") for i in range(NC1)]
            a2t = [pp.tile([128, 1], F32, tag=f"a2{i}", name=f"a2{i}") for i in range(NC2)]
            b2t = [pp.tile([128, 1], F32, tag=f"b2{i}", name=f"b2{i}") for i in range(NC2)]
            b3sb = pp.tile([C3, 1], F32, tag="b3sb", name="b3sb")
            nc.sync.dma_start(b3sb[:], b3[:])
            # per-(t,co) BN partial sums
            sx1 = [pp.tile([128, T1], F32, tag=f"sx1{i}", name=f"sx1{i}") for i in range(NC1)]
            sq1 = [pp.tile([128, T1], F32, tag=f"sq1{i}", name=f"sq1{i}") for i in range(NC1)]
            sx2 = [pp.tile([128, T2], F32, tag=f"sx2{i}", name=f"sx2{i}") for i in range(NC2)]
            sq2 = [pp.tile([128, T2], F32, tag=f"sq2{i}", name=f"sq2{i}") for i in range(NC2)]

            # ---- internal DRAM ----
            et_sh = dr.tile([L, DP, BL], F32R, name="et_sh")
            et_all = dr.tile([NCR * L, DP, BL], F32R, addr_space="Shared",
                             name="et_all")
            o1_dram = dr.tile([T1, C1P, BL], F32, name="o1_dram")
            o2_dram = dr.tile([T2, C2P, BL], F32, name="o2_dram")
            bn1_in = dr.tile([2 * C1P, 1], F32, name="bn1_in")
            bn1_out = dr.tile([2 * C1P, 1], F32, addr_space="Shared", name="bn1_out")
            bn2_in = dr.tile([2 * C2P, 1], F32, name="bn2_in")
            bn2_out = dr.tile([2 * C2P, 1], F32, addr_space="Shared", name="bn2_out")
            fn_sh = dr.tile([C3, BL], F32R, name="fn_sh")
            fn_all = dr.tile([NCR * C3, BL], F32R, addr_space="Shared", name="fn_all")

            # zero the last d-tile once (pad partitions must be 0; live rows
            # are overwritten by the transpose copies below)
            nc.vector.memset(et[NDT - 1][:, :].bitcast(F32), 0.0)

            # ================= Phase 1: gather + renorm + transpose ========
            with (
                tc.tile_pool(name="p1sb", bufs=1) as p1,
                tc.tile_pool(name="p1ps", bufs=1, space="PSUM") as p1p,
            ):
                for t in range(NT):
                    g = p1.tile([128, cfg.D], F32, tag="g", bufs=3, name="g")
                    nc.gpsimd.indirect_dma_start(
                        out=g[:], out_offset=None,
                        in_=emb[:],
                        in_offset=bass.IndirectOffsetOnAxis(
                            ap=idx_sb[:, t:t + 1], axis=0),
                    )
                    scr = p1.tile([128, cfg.D], F32, tag="scr", bufs=2, name="scr")
                    ssq = p1.tile([128, 1], F32, tag="ssq", bufs=2, name="ssq")
                    nc.scalar.activation(scr[:], g[:], AF.Square,
                                         accum_out=ssq[:, 0:1])
                    nrm = p1.tile([128, 1], F32, tag="nrm", bufs=2, name="nrm")
                    nc.scalar.sqrt(nrm[:], ssq[:])
                    den = p1.tile([128, 1], F32, tag="den", bufs=2, name="den")
                    nc.vector.tensor_scalar_add(den[:], nrm[:], EPS_RN)
                    rcp = p1.tile([128, 1], F32, tag="rcp", bufs=2, name="rcp")
                    nc.vector.reciprocal(rcp[:], den[:])
                    scl = p1.tile([128, 1], F32, tag="scl", bufs=2, name="scl")
                    nc.vector.tensor_scalar_min(scl[:], rcp[:], 1.0)
                    gs = p1.tile([128, cfg.D], F32, tag="gs", bufs=3, name="gs")
                    nc.vector.tensor_scalar_mul(gs[:], g[:], scl[:, 0:1])
                    for dt in range(NDT):
                        dlen = min(128, cfg.D - dt * 128)
                        tp = p1p.tile([128, 128], F32, tag="tp", bufs=4, name="tp")
                        nc.tensor.transpose(
                            tp[0:dlen, :], gs[:, dt * 128:dt * 128 + dlen],
                            ident[:])
                        nc.vector.tensor_copy(
                            et[dt][0:dlen, t * 128:(t + 1) * 128],
                            tp[0:dlen, :])

            # ================= Phase 2: dump ET + AllGather ================
            et_sh_t = et_sh[:].rearrange("l d b -> d l b")
            for dt in range(NDT):
                nc.sync.dma_start(
                    et_sh_t[dt * 128:(dt + 1) * 128],
                    et[dt][:].rearrange("p (l b) -> p l b", l=L))
            nc.gpsimd.collective_compute(
                "AllGather", ALU.bypass, replica_groups=RG,
                ins=[et_sh.opt()], outs=[et_all.opt()])

            # ================= Phase 4: conv1 ==============================
            with (
                tc.tile_pool(name="c1sb", bufs=1) as c1s,
                tc.tile_pool(name="c1ps", bufs=1, space="PSUM") as c1p,
            ):
                w1sb = c1s.tile([128, 5 * NC1 * C1P], F32R, tag="w1sb", name="w1sb")
                nc.sync.dma_start(w1sb[:],
                                  w1t[:].rearrange("p k c m -> p (k c m)"))
                for t1 in range(T1):
                    pairs = [(k, ci) for k in range(5)
                             if 0 <= 2 * t1 - 1 + k < L for ci in range(NC1)]
                    ps = [c1p.tile([128, BL], F32, tag=f"c1p{co}", bufs=2,
                                   name=f"c1psum{co}") for co in range(NC1)]
                    for i, (k, ci) in enumerate(pairs):
                        l = 2 * t1 - 1 + k
                        rhs = et[ci][:, l * BL:(l + 1) * BL]
                        for co in range(NC1):
                            off = (k * NC1 + ci) * C1P + co * 128
                            nc.tensor.matmul(
                                ps[co][:], w1sb[:, off:off + 128], rhs,
                                start=(i == 0), stop=(i == len(pairs) - 1),
                                skip_group_check=True)
                    for co in range(NC1):
                        nc.vector.tensor_reduce(
                            sx1[co][:, t1:t1 + 1], ps[co][:],
                            axis=mybir.AxisListType.X, op=ALU.add)
                        sqs = c1s.tile([128, BL], F32, tag="sqs", bufs=2, name="sqs")
                        nc.scalar.activation(sqs[:], ps[co][:], AF.Square,
                                             accum_out=sq1[co][:, t1:t1 + 1])
                        st = c1s.tile([128, BL], F32, tag="o1st", bufs=3, name="o1st")
                        nc.vector.tensor_copy(st[:], ps[co][:])
                        nc.sync.dma_start(
                            o1_dram[t1, co * 128:(co + 1) * 128, :], st[:])

                # BN1 stats -> DRAM -> AllReduce -> a/b
                for co in range(NC1):
                    fx = c1s.tile([128, 1], F32, tag="fx", bufs=2, name="fx")
                    nc.vector.tensor_reduce(fx[:], sx1[co][:, 0:T1],
                                            axis=mybir.AxisListType.X, op=ALU.add)
                    nc.sync.dma_start(bn1_in[co * 128:(co + 1) * 128, 0:1], fx[:])
                    fq = c1s.tile([128, 1], F32, tag="fq", bufs=2, name="fq")
                    nc.vector.tensor_reduce(fq[:], sq1[co][:, 0:T1],
                                            axis=mybir.AxisListType.X, op=ALU.add)
                    nc.sync.dma_start(
                        bn1_in[C1P + co * 128:C1P + (co + 1) * 128, 0:1], fq[:])
            nc.gpsimd.collective_compute(
                "AllReduce", ALU.add, replica_groups=RG,
                ins=[bn1_in.opt()], outs=[bn1_out.opt()])
            _bn_ab(nc, tc, cfg, bn1_out, g1b1, a1t, b1t, NC1, C1P, cfg.N1)

            # ================= Phase 5: conv2 ==============================
            with (
                tc.tile_pool(name="c2sb", bufs=1) as c2s,
                tc.tile_pool(name="c2ps", bufs=1, space="PSUM") as c2p,
            ):
                for t2 in range(T2):
                    ps = [c2p.tile([128, BL], F32, tag=f"c2p{co}", bufs=1,
                                   name=f"c2psum{co}") for co in range(NC2)]
                    i = 0
                    for k in range(5):
                        w2k = c2s.tile([128, NC1 * C2P], F32R, tag="w2k",
                                       bufs=2, name="w2k")
                        nc.sync.dma_start(
                            w2k[:],
                            w1t_slice_2d(w2t, k))
                        l1 = 2 * t2 + k
                        for ci in range(NC1):
                            tmp = c2s.tile([128, BL], F32, tag="c2tmp", bufs=2,
                                           name="c2tmp")
                            nc.sync.dma_start(
                                tmp[:], o1_dram[l1, ci * 128:(ci + 1) * 128, :])
                            r2 = c2s.tile([128, BL], F32R, tag="r2", bufs=2,
                                          name="r2")
                            nc.scalar.activation(r2[:], tmp[:], AF.Relu,
                                                 bias=b1t[ci][:, 0:1],
                                                 scale=a1t[ci][:, 0:1])
                            for co in range(NC2):
                                off = ci * C2P + co * 128
                                nc.tensor.matmul(
                                    ps[co][:], w2k[:, off:off + 128], r2[:],
                                    start=(i == 0), stop=(i == 14),
                                    skip_group_check=True)
                            i += 1
                    for co in range(NC2):
                        nc.vector.tensor_reduce(
                            sx2[co][:, t2:t2 + 1], ps[co][:],
                            axis=mybir.AxisListType.X, op=ALU.add)
                        sqs2 = c2s.tile([128, BL], F32, tag="sqs2", bufs=2,
                                        name="sqs2")
                        nc.scalar.activation(sqs2[:], ps[co][:], AF.Square,
                                             accum_out=sq2[co][:, t2:t2 + 1])
                        st2 = c2s.tile([128, BL], F32, tag="o2st", bufs=3,
                                       name="o2st")
                        nc.vector.tensor_copy(st2[:], ps[co][:])
                        nc.sync.dma_start(
                            o2_dram[t2, co * 128:(co + 1) * 128, :], st2[:])
                for co in range(NC2):
                    fx2 = c2s.tile([128, 1], F32, tag="fx2", bufs=2, name="fx2")
                    nc.vector.tensor_reduce(fx2[:], sx2[co][:, 0:T2],
                                            axis=mybir.AxisListType.X, op=ALU.add)
                    nc.sync.dma_start(bn2_in[co * 128:(co + 1) * 128, 0:1], fx2[:])
                    fq2 = c2s.tile([128, 1], F32, tag="fq2", bufs=2, name="fq2")
                    nc.vector.tensor_reduce(fq2[:], sq2[co][:, 0:T2],
                                            axis=mybir.AxisListType.X, op=ALU.add)
                    nc.sync.dma_start(
                        bn2_in[C2P + co * 128:C2P + (co + 1) * 128, 0:1], fq2[:])
            nc.gpsimd.collective_compute(
                "AllReduce", ALU.add, replica_groups=RG,
                ins=[bn2_in.opt()], outs=[bn2_out.opt()])
            _bn_ab(nc, tc, cfg, bn2_out, g2b2, a2t, b2t, NC2, C2P, cfg.N2)

            # ================= Phase 6: conv3 + tanh + normalize ===========
            with (
                tc.tile_pool(name="c3sb", bufs=1) as c3s,
                tc.tile_pool(name="c3ps", bufs=1, space="PSUM") as c3p,
            ):
                w3sb = c3s.tile([128, 5 * NC2 * C3], F32R, tag="w3sb", name="w3sb")
                nc.sync.dma_start(w3sb[:],
                                  w3t[:].rearrange("p k c m -> p (k c m)"))
                ps3 = c3p.tile([128, BL], F32, tag="c3p", bufs=1, name="c3psum")
                i = 0
                for k in range(5):
                    for ci in range(NC2):
                        tmp3 = c3s.tile([128, BL], F32, tag="c3tmp", bufs=3,
                                        name="c3tmp")
                        nc.sync.dma_start(
                            tmp3[:], o2_dram[k, ci * 128:(ci + 1) * 128, :])
                        r3 = c3s.tile([128, BL], F32R, tag="r3", bufs=3, name="r3")
                        nc.scalar.activation(r3[:], tmp3[:], AF.Relu,
                                             bias=b2t[ci][:, 0:1],
                                             scale=a2t[ci][:, 0:1])
                        off = (k * NC2 + ci) * C3
                        nc.tensor.matmul(ps3[0:C3, :], w3sb[:, off:off + C3],
                                         r3[:], start=(i == 0), stop=(i == 24),
                                         skip_group_check=True)
                        i += 1
                ft = c3s.tile([C3, BL], F32, tag="ft", name="ft")
                nc.scalar.activation(ft[:], ps3[0:C3, :], AF.Tanh,
                                     bias=b3sb[:, 0:1])
                for m in range(MB):
                    tph = c3p.tile([128, 128], F32, tag="tph", bufs=2, name="tph")
                    nc.tensor.transpose(tph[:, 0:C3],
                                        ft[:, m * 128:(m + 1) * 128],
                                        ident[0:C3, 0:C3])
                    hs = c3s.tile([128, C3], F32, tag="hs", bufs=2, name="hs")
                    nc.vector.tensor_copy(hs[:], tph[:, 0:C3])
                    nc.sync.dma_start(h_out[m * 128:(m + 1) * 128, :], hs[:])
                    scr2 = c3s.tile([128, C3], F32, tag="scr2", bufs=2, name="scr2")
                    sqh = c3s.tile([128, 1], F32, tag="sqh", bufs=2, name="sqh")
                    nc.scalar.activation(scr2[:], hs[:], AF.Square,
                                         accum_out=sqh[:, 0:1])
                    nrh = c3s.tile([128, 1], F32, tag="nrh", bufs=2, name="nrh")
                    nc.scalar.sqrt(nrh[:], sqh[:])
                    rch = c3s.tile([128, 1], F32, tag="rch", bufs=2, name="rch")
                    nc.vector.reciprocal(rch[:], nrh[:])
                    fnb = c3s.tile([128, C3], F32, tag="fnb", bufs=2, name="fnb")
                    nc.vector.tensor_scalar_mul(fnb[:], hs[:], rch[:, 0:1])
                    tpf = c3p.tile([128, 128], F32, tag="tpf", bufs=2, name="tpf")
                    nc.tensor.transpose(tpf[0:C3, :], fnb[:], ident[:])
                    nc.vector.tensor_copy(fnT[0:C3, m * 128:(m + 1) * 128],
                                          tpf[0:C3, :])
                nc.sync.dma_start(fn_sh[:], fnT[0:C3, :])
            nc.gpsimd.collective_compute(
                "AllGather", ALU.bypass, replica_groups=RG,
                ins=[fn_sh.opt()], outs=[fn_all.opt()])

            # ================= Phase 7: gram_h =============================
            with (
                tc.tile_pool(name="ghsb", bufs=1) as ghs,
                tc.tile_pool(name="ghps", bufs=1, space="PSUM") as ghp,
            ):
                for jb in range(NCR):
                    fr = ghs.tile([C3, BL], F32R, tag="fr", bufs=2, name="fr")
                    nc.sync.dma_start(fr[:], fn_all[jb * C3:(jb + 1) * C3, :])
                    for m in range(MB):
                        ph = ghp.tile([128, BL], F32, tag=f"ph{m % 2}", bufs=2,
                                      name=f"ph{m % 2}")
                        nc.tensor.matmul(ph[:], fnT[0:C3, m * 128:(m + 1) * 128],
                                         fr[:], start=True, stop=True)
                        sh = ghs.tile([128, BL], F32, tag="sh", bufs=4, name="sh")
                        nc.vector.tensor_copy(sh[:], ph[:])
                        nc.sync.dma_start(
                            g_h_blk[m * 128:(m + 1) * 128,
                                    jb * BL:(jb + 1) * BL], sh[:])

            # ================= Phase 8: gram_in ============================
            with (
                tc.tile_pool(name="gisb", bufs=1) as gis,
                tc.tile_pool(name="gips", bufs=1, space="PSUM") as gip,
            ):
                n_kt = L * NDT
                for jb in range(NCR):
                    psg = [gip.tile([128, BL], F32, tag=f"gp{m}", bufs=2,
                                    name=f"gpsum{m}") for m in range(MB)]
                    kt = 0
                    for l in range(L):
                        for dt in range(NDT):
                            r = gis.tile([128, BL], F32R, tag="gr", bufs=4,
                                         name="gr")
                            nc.sync.dma_start(
                                r[:],
                                et_all[jb * L + l, dt * 128:(dt + 1) * 128, :])
                            for m in range(MB):
                                lo = l * BL + m * 128
                                nc.tensor.matmul(
                                    psg[m][:], et[dt][:, lo:lo + 128], r[:],
                                    start=(kt == 0), stop=(kt == n_kt - 1),
                                    skip_group_check=True)
                            kt += 1
                    for m in range(MB):
                        sg = gis.tile([128, BL], F32, tag="sg", bufs=4, name="sg")
                        nc.vector.tensor_copy(sg[:], psg[m][:])
                        nc.sync.dma_start(
                            g_in_blk[m * 128:(m + 1) * 128,
                                     jb * BL:(jb + 1) * BL], sg[:])

    nc.compile()
    return nc


def w1t_slice_2d(w2t, k):
    # w2t dram [128, 5, NC1, C2P] -> [128, NC1*C2P] slice at tap k
    return w2t[:, k].rearrange("p c m -> p (c m)")


def _bn_ab(nc, tc, cfg, bn_out, gb_dram, a_tiles, b_tiles, NCt, CP, N):
    """Compute BN scale a = gamma*rsqrt(var+eps), shift b = beta - mean*a
    from the all-reduced [2*CP,1] sums tensor."""
    with tc.tile_pool(name="bnab", bufs=1) as bp:
        for co in range(NCt):
            sx = bp.tile([128, 1], F32, tag="sx", bufs=2, name="bnsx")
            nc.sync.dma_start(sx[:], bn_out[co * 128:(co + 1) * 128, 0:1])
            sq = bp.tile([128, 1], F32, tag="sq", bufs=2, name="bnsq")
            nc.sync.dma_start(sq[:], bn_out[CP + co * 128:CP + (co + 1) * 128, 0:1])
            gb = bp.tile([128, 2], F32, tag="gb", bufs=2, name="bngb")
            nc.sync.dma_start(gb[:], gb_dram[co])
            mean = bp.tile([128, 1], F32, tag="mean", bufs=2, name="bnmean")
            nc.vector.tensor_scalar_mul(mean[:], sx[:], 1.0 / N)
            e2 = bp.tile([128, 1], F32, tag="e2", bufs=2, name="bne2")
            nc.vector.tensor_scalar_mul(e2[:], sq[:], 1.0 / N)
            var = bp.tile([128, 1], F32, tag="var", bufs=2, name="bnvar")
            nc.vector.tensor_tensor(var[:], mean[:], mean[:], op=ALU.mult)
            nc.vector.tensor_tensor(var[:], e2[:], var[:], op=ALU.subtract)
            vare = bp.tile([128, 1], F32, tag="vare", bufs=2, name="bnvare")
            nc.vector.tensor_scalar_add(vare[:], var[:], EPS_BN)
            sd = bp.tile([128, 1], F32, tag="sd", bufs=2, name="bnsd")
            nc.scalar.sqrt(sd[:], vare[:])
            inv = bp.tile([128, 1], F32, tag="inv", bufs=2, name="bninv")
            nc.vector.reciprocal(inv[:], sd[:])
            nc.vector.tensor_tensor(a_tiles[co][:], inv[:], gb[:, 0:1],
                                    op=ALU.mult)
            tmp = bp.tile([128, 1], F32, tag="tmpb", bufs=2, name="bntmp")
            nc.vector.tensor_tensor(tmp[:], mean[:], a_tiles[co][:], op=ALU.mult)
            nc.vector.tensor_tensor(b_tiles[co][:], gb[:, 1:2], tmp[:],
                                    op=ALU.subtract)


# ======================= host-side prep / post =======================

def prep_in_maps(cfg: Cfg, x, emb, w1, b1, w2, b2, w3, b3, g1, be1, g2, be2):
    L, BL, NT = cfg.L, cfg.BL, cfg.NT

    xs = np.asarray(x).astype(np.int32)              # [B, L]
    emb_f = np.ascontiguousarray(np.asarray(emb, dtype=np.float32))

    def wt_prep(w, cinp, coutp):
        # w: [cout, cin, 5] -> [128, 5, cinp//128, coutp] zero-padded,
        # element [r, k, ci, co] = w[co, ci*128+r, k]
        cout, cin, K = w.shape
        wp = np.zeros((coutp, cinp, K), np.float32)
        wp[:cout, :cin, :] = np.asarray(w, dtype=np.float32)
        # -> [cin_tile r, k, ci, co]
        wt = wp.reshape(coutp, cinp // 128, 128, K).transpose(2, 3, 1, 0)
        return np.ascontiguousarray(wt)

    w1t = wt_prep(w1, cfg.C1P, cfg.C1P)              # [128, 5, 3, 384]
    w2t = wt_prep(w2, cfg.C1P, cfg.C2P)              # [128, 5, 3, 640]
    w3t = wt_prep(w3, cfg.C2P, cfg.C3)               # [128, 5, 5, 100]

    def gb_prep(g, be, cp):
        gp = np.zeros((cp, 2), np.float32)
        gp[:len(g), 0] = np.asarray(g, dtype=np.float32)
        gp[:len(be), 1] = np.asarray(be, dtype=np.float32)
        return np.ascontiguousarray(gp.reshape(cp // 128, 128, 2))

    g1b1 = gb_prep(g1, be1, cfg.C1P)
    g2b2 = gb_prep(g2, be2, cfg.C2P)
    b3h = np.ascontiguousarray(np.asarray(b3, dtype=np.float32).reshape(cfg.C3, 1))

    in_maps = []
    for c in range(cfg.ncores):
        xb = xs[c * BL:(c + 1) * BL, :]              # [BL, L]
        flat = np.ascontiguousarray(xb.T).reshape(-1)  # l-major tokens
        x_til = np.ascontiguousarray(flat.reshape(NT, 128).T)  # [128, NT]
        in_maps.append({
            "x_til": x_til, "emb": emb_f,
            "w1t": w1t, "w2t": w2t, "w3t": w3t,
            "g1b1": g1b1, "g2b2": g2b2, "b3": b3h,
        })
    return in_maps


def postprocess(cfg: Cfg, results):
    h = np.concatenate([r["h_out"] for r in results], axis=0)       # [B, 100]
    g_in = np.concatenate([r["g_in_blk"] for r in results], axis=0)  # [B, B]
    g_h = np.concatenate([r["g_h_blk"] for r in results], axis=0)
    iu, ju = np.triu_indices(cfg.B, k=1)
    input_pws = np.ascontiguousarray(g_in[iu, ju])
    hidden_pws = np.ascontiguousarray(g_h[iu, ju])
    return h[:, :, None].astype(np.float32), input_pws.astype(np.float32), \
        hidden_pws.astype(np.float32)


_CACHED = {}


def kernel(**inputs):
    cfg = Cfg()
    if "nc" not in _CACHED:
        _CACHED["nc"] = build(cfg)
    nc = _CACHED["nc"]
    in_maps = prep_in_maps(cfg, **{k: inputs[k] for k in
                                   ("x", "emb", "w1", "b1", "w2", "b2", "w3",
                                    "b3", "g1", "be1", "g2", "be2")})
    res = bass_utils.run_bass_kernel_spmd(
        nc, in_maps, core_ids=list(range(cfg.ncores)))
    return postprocess(cfg, res.results)
